# revision 21
# baseline (speedup 1.0000x reference)
"""BlockTucker kernel for TRN2, 8 NeuronCores, data-parallel over batch.

Model (per reference):
    h0 = (x0 @ W0.T + b0).reshape(B, C, S)          B=8192 DIN=2048 MM=1600
    h1 = (x1 @ W1.T + b1).reshape(B, C, S)          C=20 chunks, S=80
    z[b,c,q] = sum_{s,t} h0[b,c,s] Wb[c,q,s,t] h1[b,c,t] + bb[c,q]
    z = signed_sqrt(z); z = z / max(||z||_chunk, eps); out = z @ Wout.T + bout

v2 dataflow (BL = 1024 rows/core; weights host-pretransposed + bf16):
  stage A0 (PE): h0T[m, b] = W0T.T @ x0T + b0  -> DRAM bf16 (lhsT for middle)
  stage A1 (PE): h1b[b, m] = x1T.T @ W1T       -> SBUF bf16 (+b1 via DVE)
  middle, per chunk c / batch-tile bt (q-major free dim):
      PE:   Y2[b,(q,t)] = h0c[s,b].T @ WbS[c][s,(q,t)]   (K=80, bf16)
      ACT:  evacuate PSUM -> y2 bf16 (5 slices, 3-bank-wide)
      DVE:  gate g = y2 * h1b bcast (2x bf16); tree t1 (+t2 on parity)
      Pool: tree t3 (+t2 on parity) + final reduce -> zst bf16 -> zb DRAM
  tail (DVE/ACT/Pool): +bb, signed-sqrt via rsqrt, per-chunk L2 norm -> zn
  out-proj (PE): znT on-chip transposes; out = znT.T @ WoutT + bout
"""

import numpy as np

BL = 1024
DIN = 2048
MM = 1600
C, S = 20, 80
OUT = 3000
NCORES = 8
QT = S * S
NKT = DIN // 128            # 16
NMT = 13                    # 12x128 + 64
NBT = BL // 128             # 8
NOG = 6                     # out column groups: 5x512 + 440
EPS = 1e-12

_CACHE = {}


def _msz(mt):
    return 128 if mt < NMT - 1 else MM - 128 * (NMT - 1)


def _osz(og):
    return 512 if og < NOG - 1 else OUT - 512 * (NOG - 1)


def _build():
    import concourse.bass as bass
    import concourse.mybir as mybir
    import concourse.tile as tile
    from concourse.masks import make_identity
    from contextlib import ExitStack

    f32 = mybir.dt.float32
    bf16 = mybir.dt.bfloat16
    AF = mybir.ActivationFunctionType
    ALU = mybir.AluOpType
    AX = mybir.AxisListType

    nc = bass.Bass()

    x0T = nc.declare_dram_parameter("x0T", [DIN, BL], bf16, isOutput=False)
    x1T = nc.declare_dram_parameter("x1T", [DIN, BL], bf16, isOutput=False)
    W0T = nc.declare_dram_parameter("W0T", [DIN, MM], bf16, isOutput=False)
    W1T = nc.declare_dram_parameter("W1T", [DIN, MM], bf16, isOutput=False)
    b0 = nc.declare_dram_parameter("b0", [MM], f32, isOutput=False)
    b1 = nc.declare_dram_parameter("b1", [MM], bf16, isOutput=False)
    WbS = nc.declare_dram_parameter("WbS", [C, S, QT], bf16, isOutput=False)
    bb = nc.declare_dram_parameter("bb", [C, S], bf16, isOutput=False)
    WoT = nc.declare_dram_parameter("WoT", [MM, OUT], bf16, isOutput=False)
    bout = nc.declare_dram_parameter("bout", [OUT], bf16, isOutput=False)
    out = nc.declare_dram_parameter("out", [BL, OUT], f32, isOutput=True)

    h0T_d = nc.dram_tensor("h0T_d", [MM, BL], bf16)
    zb_d = nc.dram_tensor("zb_d", [BL, MM], bf16)
    zn_d = nc.dram_tensor("zn_d", [BL, MM], bf16)

    # middle PSUM fill plan: 13 N-slices (12x512 + 256) packed into 7
    # [128,<=1024] psum tiles (2 banks each, triple-buffered = 6 banks)
    FILLS = [(0, (512, 512)), (1024, (512, 512)), (2048, (512, 512)),
             (3072, (512, 512)), (4096, (512, 512)), (5120, (512, 512)),
             (6144, (256,))]

    with tile.TileContext(nc) as tc:
        with ExitStack() as top:
            const = top.enter_context(tc.tile_pool(name="const", bufs=1))
            h1p = top.enter_context(tc.tile_pool(name="h1b", bufs=1))
            ps_aux = top.enter_context(
                tc.tile_pool(name="ps_aux", bufs=2, space="PSUM"))

            identb = const.tile([128, 128], bf16)
            identf = const.tile([128, 128], f32)
            make_identity(nc, identf)
            nc.vector.tensor_copy(identb[:], identf[:])

            # b0 as per-partition bias columns: col mt holds m = mt*128 + p
            b0sb = const.tile([128, NMT], f32)
            nc.sync.dma_start(
                out=b0sb[:, : NMT - 1],
                in_=b0[: 128 * (NMT - 1)].rearrange("(j p) -> p j", p=128),
            )
            nc.sync.dma_start(
                out=b0sb[: _msz(NMT - 1), NMT - 1 : NMT],
                in_=b0[128 * (NMT - 1) :].unsqueeze(1),
            )
            b1rep = const.tile([128, MM], bf16)
            nc.sync.dma_start(
                out=b1rep[:], in_=b1[:].unsqueeze(0).broadcast_to([128, MM])
            )
            bbrep = const.tile([128, MM], bf16)
            nc.sync.dma_start(
                out=bbrep[:],
                in_=bb[:].rearrange("c q -> (c q)").unsqueeze(0).broadcast_to(
                    [128, MM]
                ),
            )
            borep = const.tile([128, OUT], bf16)
            nc.sync.dma_start(
                out=borep[:], in_=bout[:].unsqueeze(0).broadcast_to([128, OUT])
            )

            h1b = h1p.tile([128, NBT, MM], bf16)

            # ==== phase 1: A1 (h1b) + A0 (h0T) + middle, fully interleaved ===
            MSL = [(i * 256, 256) for i in range(6)] + [(1536, 64)]
            with ExitStack() as mctx:
                ps_mid = mctx.enter_context(
                    tc.tile_pool(name="ps_mid", bufs=3, space="PSUM"))
                xbp = mctx.enter_context(tc.tile_pool(name="a1x", bufs=1))
                w1p = mctx.enter_context(tc.tile_pool(name="a1w", bufs=2))
                e1p = mctx.enter_context(tc.tile_pool(name="a1e", bufs=2))
                xap = mctx.enter_context(tc.tile_pool(name="a0x", bufs=1))
                w0p = mctx.enter_context(tc.tile_pool(name="a0w", bufs=2))
                e0p = mctx.enter_context(tc.tile_pool(name="a0e", bufs=2))
                wbp = mctx.enter_context(tc.tile_pool(name="wb", bufs=2))
                h0p = mctx.enter_context(tc.tile_pool(name="h0c", bufs=2))
                y2p = mctx.enter_context(tc.tile_pool(name="y2", bufs=2))
                t2p = mctx.enter_context(tc.tile_pool(name="t2", bufs=2))
                t3p = mctx.enter_context(tc.tile_pool(name="t3", bufs=2))
                zp = mctx.enter_context(tc.tile_pool(name="zst", bufs=3))

                xb = xbp.tile([128, NKT, BL], bf16)
                nc.sync.dma_start(
                    out=xb[:], in_=x1T[:].rearrange("(k p) b -> p k b", p=128)
                )
                xa = xap.tile([128, NKT, BL], bf16)
                nc.sync.dma_start(
                    out=xa[:], in_=x0T[:].rearrange("(k p) b -> p k b", p=128)
                )

                def emit_a1_ms(msi):
                    mo, mw = MSL[msi]
                    w1 = w1p.tile([128, NKT, 256], bf16, tag="w1")
                    nc.sync.dma_start(
                        out=w1[:, :, :mw],
                        in_=W1T[:, mo : mo + mw].rearrange(
                            "(k p) m -> p k m", p=128
                        ),
                    )
                    for bt in range(NBT):
                        ps = ps_aux.tile([128, 512], f32, tag="psa")
                        for k in range(NKT):
                            nc.tensor.matmul(
                                ps[:, :mw],
                                lhsT=xb[:, k, bt * 128 : (bt + 1) * 128],
                                rhs=w1[:, k, :mw],
                                start=(k == 0),
                                stop=(k == NKT - 1),
                            )
                        ev = e1p.tile([128, 256], bf16, tag="e1")
                        nc.scalar.activation(ev[:, :mw], ps[:, :mw], AF.Identity)
                        nc.vector.tensor_tensor(
                            out=h1b[:, bt, mo : mo + mw],
                            in0=ev[:, :mw],
                            in1=b1rep[:, mo : mo + mw],
                            op=ALU.add,
                        )

                def emit_a0_mt(mt):
                    ms = _msz(mt)
                    msl = slice(mt * 128, mt * 128 + ms)
                    w0 = w0p.tile([128, NKT, 128], bf16, tag="w0")
                    nc.sync.dma_start(
                        out=w0[:, :, :ms],
                        in_=W0T[:, msl].rearrange("(k p) m -> p k m", p=128),
                    )
                    for hh in range(2):
                        hsl = slice(hh * 512, (hh + 1) * 512)
                        ps = ps_aux.tile([128, 512], f32, tag="psa")
                        for k in range(NKT):
                            nc.tensor.matmul(
                                ps[:ms, :],
                                lhsT=w0[:, k, :ms],
                                rhs=xa[:, k, hsl],
                                start=(k == 0),
                                stop=(k == NKT - 1),
                            )
                        ev = e0p.tile([128, 512], bf16, tag="e0")
                        nc.scalar.activation(
                            ev[:ms, :], ps[:ms, :], AF.Identity,
                            bias=b0sb[:ms, mt : mt + 1],
                        )
                        nc.sync.dma_start(out=h0T_d[msl, hsl], in_=ev[:ms, :])

                def emit_chunk(c):
                    csl = slice(c * S, (c + 1) * S)
                    wbs = wbp.tile([S, QT], bf16, tag="wbs")
                    nc.sync.dma_start(out=wbs[:], in_=WbS[c])
                    h0c = h0p.tile([S, BL], bf16, tag="h0c")
                    nc.sync.dma_start(out=h0c[:], in_=h0T_d[csl, :])
                    for bt in range(NBT):
                        bsl = slice(bt * 128, (bt + 1) * 128)
                        y2 = y2p.tile([128, QT], bf16, tag="y2")
                        for off, ws in FILLS:
                            fw = sum(ws)
                            ps = ps_mid.tile([128, 1024], f32, tag="mid")
                            o = 0
                            for w in ws:
                                nc.tensor.matmul(
                                    ps[:, o : o + w],
                                    lhsT=h0c[:, bsl],
                                    rhs=wbs[:, off + o : off + o + w],
                                    start=True,
                                    stop=True,
                                )
                                o += w
                            nc.scalar.activation(
                                y2[:, off : off + fw], ps[:, :fw], AF.Identity
                            )
                        y23 = y2[:].rearrange("p (q t) -> p q t", t=S)
                        nc.vector.tensor_tensor(
                            out=y23,
                            in0=y23,
                            in1=h1b[:, bt, csl].unsqueeze(1).broadcast_to(
                                [128, S, S]
                            ),
                            op=ALU.mult,
                        )
                        nc.vector.tensor_tensor(
                            out=y23[:, :, :40], in0=y23[:, :, :40],
                            in1=y23[:, :, 40:], op=ALU.add,
                        )
                        t2 = t2p.tile([128, S, 20], bf16, tag="t2")
                        nc.gpsimd.tensor_tensor(
                            out=t2[:], in0=y23[:, :, :20], in1=y23[:, :, 20:40],
                            op=ALU.add,
                        )
                        t3 = t3p.tile([128, S, 10], bf16, tag="t3")
                        nc.gpsimd.tensor_tensor(
                            out=t3[:], in0=t2[:, :, :10], in1=t2[:, :, 10:],
                            op=ALU.add,
                        )
                        zst = zp.tile([128, S], bf16, tag="zst")
                        with nc.allow_low_precision(
                            reason="80-term sum accumulates fp32 in-engine"
                        ):
                            nc.vector.tensor_reduce(
                                out=zst[:], in_=t3[:], axis=AX.X, op=ALU.add
                            )
                        nc.sync.dma_start(out=zb_d[bsl, csl], in_=zst[:])

                # chunk c is emittable once A0 covers rows 80c+79 (mt) and A1
                # covers cols 80c+79 (ms)
                def ms_hi(c):
                    return min(len(MSL) - 1, (80 * c + 79) // 256)

                def mt_hi(c):
                    return (80 * c + 79) // 128

                done = 0
                ms_done = 0
                emit_a1_ms(0)
                ms_done = 1
                for mt in range(NMT):
                    emit_a0_mt(mt)
                    ready = min(C, (128 * (mt + 1)) // 80)
                    while done < ready:
                        if ms_hi(done) >= ms_done:
                            emit_a1_ms(ms_done)
                            ms_done += 1
                        emit_chunk(done)
                        done += 1
                while ms_done < len(MSL):
                    emit_a1_ms(ms_done)
                    ms_done += 1
                while done < C:
                    emit_chunk(done)
                    done += 1

            # ==== phase 2: tail + znT + out-proj, per batch-tile ====
            with ExitStack() as octx:
                ps_out = octx.enter_context(
                    tc.tile_pool(name="ps_out", bufs=2, space="PSUM"))
                zlp = octx.enter_context(tc.tile_pool(name="tl", bufs=2))
                znpp = octx.enter_context(tc.tile_pool(name="zn", bufs=2))
                sp = octx.enter_context(tc.tile_pool(name="tls", bufs=2))
                znp = octx.enter_context(tc.tile_pool(name="znT", bufs=1))
                wop = octx.enter_context(tc.tile_pool(name="wo", bufs=1))
                evp = octx.enter_context(tc.tile_pool(name="oev", bufs=3))

                # all 6 Wout column groups resident (bf16, 80KB)
                wos = wop.tile([128, NOG, NMT, 512], bf16)
                for og in range(NOG):
                    ow = _osz(og)
                    osl = slice(og * 512, og * 512 + ow)
                    nc.sync.dma_start(
                        out=wos[:, og, : NMT - 1, :ow],
                        in_=WoT[: 128 * (NMT - 1), osl].rearrange(
                            "(k p) o -> p k o", p=128
                        ),
                    )
                    nc.sync.dma_start(
                        out=wos[: _msz(NMT - 1), og, NMT - 1, :ow],
                        in_=WoT[128 * (NMT - 1) :, osl],
                    )

                znT = znp.tile([128, NMT, 128], bf16)  # one bt at a time
                for bt in range(NBT):
                    bsl = slice(bt * 128, (bt + 1) * 128)
                    zt = zlp.tile([128, MM], bf16, tag="zt")
                    nc.sync.dma_start(out=zt[:], in_=zb_d[bsl, :])
                    zbb = zlp.tile([128, MM], bf16, tag="zbb")
                    nc.vector.tensor_tensor(
                        out=zbb[:], in0=zt[:], in1=bbrep[:], op=ALU.add
                    )
                    sab = zlp.tile([128, MM], bf16, tag="zt")
                    nc.scalar.activation(sab[:], zbb[:], AF.Abs)
                    sq = zlp.tile([128, MM], bf16, tag="sq")
                    nc.scalar.activation(sq[:], sab[:], AF.Sqrt)
                    sgn = zlp.tile([128, MM], bf16, tag="sgn")
                    nc.scalar.activation(sgn[:], zbb[:], AF.Sign)
                    ss = zlp.tile([128, MM], bf16, tag="ss")
                    nc.vector.tensor_tensor(
                        out=ss[:], in0=sgn[:], in1=sq[:], op=ALU.mult
                    )
                    nsq = sp.tile([128, C], f32, tag="nsq")
                    nc.vector.tensor_reduce(
                        out=nsq[:],
                        in_=zbb[:].rearrange("p (c q) -> p c q", q=S),
                        axis=AX.X,
                        op=ALU.add,
                        apply_absolute_value=True,
                    )
                    nrm = sp.tile([128, C], f32, tag="nrm")
                    nc.scalar.activation(nrm[:], nsq[:], AF.Sqrt)
                    nrmc = sp.tile([128, C], f32, tag="nrmc")
                    nc.vector.tensor_scalar_max(
                        out=nrmc[:], in0=nrm[:], scalar1=EPS
                    )
                    inv = sp.tile([128, C], f32, tag="inv")
                    nc.vector.reciprocal(inv[:], nrmc[:])
                    zn = znpp.tile([128, MM], bf16, tag="zn")
                    nc.vector.tensor_tensor(
                        out=zn[:].rearrange("p (c q) -> p c q", q=S),
                        in0=ss[:].rearrange("p (c q) -> p c q", q=S),
                        in1=inv[:].unsqueeze(2).broadcast_to([128, C, S]),
                        op=ALU.mult,
                    )
                    # transpose this bt into znT, then its 6 out-proj groups
                    for kq in range(4):
                        pst = ps_out.tile([128, 512], bf16, tag="pst")
                        nw = min(4, NMT - kq * 4)
                        for i in range(nw):
                            k = kq * 4 + i
                            ks = _msz(k)
                            nc.tensor.transpose(
                                pst[:ks, i * 128 : i * 128 + 128],
                                zn[:, k * 128 : k * 128 + ks],
                                identb[:],
                            )
                        nc.scalar.activation(
                            znT[:, kq * 4 : kq * 4 + nw, :],
                            pst[:, : nw * 128].rearrange(
                                "p (k b) -> p k b", b=128
                            ),
                            AF.Identity,
                        )
                    for og in range(NOG):
                        ow = _osz(og)
                        osl = slice(og * 512, og * 512 + ow)
                        ps = ps_out.tile([128, 512], f32, tag="acc")
                        for k in range(NMT):
                            ks = _msz(k)
                            nc.tensor.matmul(
                                ps[:, :ow],
                                lhsT=znT[:ks, k, :],
                                rhs=wos[:ks, og, k, :ow],
                                start=(k == 0),
                                stop=(k == NMT - 1),
                            )
                        ev = evp.tile([128, 512], f32, tag="oev")
                        nc.vector.tensor_tensor(
                            out=ev[:, :ow], in0=ps[:, :ow], in1=borep[:, osl],
                            op=ALU.add,
                        )
                        nc.sync.dma_start(out=out[bsl, osl], in_=ev[:, :ow])

    _split_excess_waits(nc, cap=4)
    return nc


def _split_excess_waits(nc, cap=4):
    """Walrus rejects instructions with too many sync waits. Move excess
    waits onto NoOps spliced just before the instruction on the same engine
    queue (the sequencer executes them in order, so semantics are identical).
    """
    import concourse.mybir as mybir
    import bass_rust

    n = 0
    for f in nc.m.functions:
        for blk in f.blocks:
            out = []
            changed = False
            for inst in blk.instructions:
                si = getattr(inst, "sync_info", None)
                waits = list(si.on_wait) if si is not None and si.on_wait else []
                icap = 2 if inst.opcode == "EventSemaphore" else 1
                if len(waits) > icap:
                    excess, keep = waits[:-icap], waits[-icap:]
                    for w in excess:
                        nop = mybir.InstNoOp(
                            name=f"{inst.name}-wsplit{n}", ins=[], outs=[]
                        )
                        n += 1
                        nop.engine = inst.engine
                        nop.sync_info = bass_rust.SyncInfo(
                            on_wait=[w], on_update=[]
                        )
                        out.append(nop)
                    inst.sync_info = bass_rust.SyncInfo(
                        on_wait=keep, on_update=list(si.on_update or [])
                    )
                    changed = True
                out.append(inst)
            if changed:
                blk.instructions = out
    return nc


def _get_nc():
    if "nc" not in _CACHE:
        _CACHE["nc"] = _build()
    return _CACHE["nc"]


def _shard_inputs(inputs):
    import ml_dtypes

    bf = ml_dtypes.bfloat16
    full = {k: np.asarray(v, dtype=np.float32) for k, v in inputs.items()}
    shared = {
        "W0T": np.ascontiguousarray(full["W0"].T.astype(bf)),
        "W1T": np.ascontiguousarray(full["W1"].T.astype(bf)),
        "WbS": np.ascontiguousarray(
            full["Wb"].transpose(0, 2, 1, 3).reshape(C, S, QT).astype(bf)
        ),
        "WoT": np.ascontiguousarray(full["Wout"].T.astype(bf)),
        "b0": np.ascontiguousarray(full["b0"]),
        "b1": np.ascontiguousarray(full["b1"].astype(bf)),
        "bb": np.ascontiguousarray(full["bb"].astype(bf)),
        "bout": np.ascontiguousarray(full["bout"].astype(bf)),
    }
    rows = full["x0"].shape[0] // NCORES
    in_maps = []
    for i in range(NCORES):
        m = dict(shared)
        m["x0T"] = np.ascontiguousarray(
            full["x0"][i * rows : (i + 1) * rows].T.astype(bf)
        )
        m["x1T"] = np.ascontiguousarray(
            full["x1"][i * rows : (i + 1) * rows].T.astype(bf)
        )
        in_maps.append(m)
    return in_maps


def kernel(**inputs):
    from concourse.bass_utils import run_bass_kernel_spmd

    nc = _get_nc()
    in_maps = _shard_inputs(inputs)
    res = run_bass_kernel_spmd(nc, in_maps, list(range(NCORES)))
    return np.concatenate([res.results[i]["out"] for i in range(NCORES)], axis=0)


# revision 22
# speedup vs baseline: 1.0204x; 1.0204x over previous
"""BlockTucker kernel for TRN2, 8 NeuronCores, data-parallel over batch.

Model (per reference):
    h0 = (x0 @ W0.T + b0).reshape(B, C, S)          B=8192 DIN=2048 MM=1600
    h1 = (x1 @ W1.T + b1).reshape(B, C, S)          C=20 chunks, S=80
    z[b,c,q] = sum_{s,t} h0[b,c,s] Wb[c,q,s,t] h1[b,c,t] + bb[c,q]
    z = signed_sqrt(z); z = z / max(||z||_chunk, eps); out = z @ Wout.T + bout

v2 dataflow (BL = 1024 rows/core; weights host-pretransposed + bf16):
  stage A0 (PE): h0T[m, b] = W0T.T @ x0T + b0  -> DRAM bf16 (lhsT for middle)
  stage A1 (PE): h1b[b, m] = x1T.T @ W1T       -> SBUF bf16 (+b1 via DVE)
  middle, per chunk c / batch-tile bt (q-major free dim):
      PE:   Y2[b,(q,t)] = h0c[s,b].T @ WbS[c][s,(q,t)]   (K=80, bf16)
      ACT:  evacuate PSUM -> y2 bf16 (5 slices, 3-bank-wide)
      DVE:  gate g = y2 * h1b bcast (2x bf16); tree t1 (+t2 on parity)
      Pool: tree t3 (+t2 on parity) + final reduce -> zst bf16 -> zb DRAM
  tail (DVE/ACT/Pool): +bb, signed-sqrt via rsqrt, per-chunk L2 norm -> zn
  out-proj (PE): znT on-chip transposes; out = znT.T @ WoutT + bout
"""

import numpy as np

BL = 1024
DIN = 2048
MM = 1600
C, S = 20, 80
OUT = 3000
NCORES = 8
QT = S * S
NKT = DIN // 128            # 16
NMT = 13                    # 12x128 + 64
NBT = BL // 128             # 8
NOG = 6                     # out column groups: 5x512 + 440
EPS = 1e-12

_CACHE = {}


def _msz(mt):
    return 128 if mt < NMT - 1 else MM - 128 * (NMT - 1)


def _osz(og):
    return 512 if og < NOG - 1 else OUT - 512 * (NOG - 1)


def _build():
    import concourse.bass as bass
    import concourse.mybir as mybir
    import concourse.tile as tile
    from concourse.masks import make_identity
    from contextlib import ExitStack

    f32 = mybir.dt.float32
    bf16 = mybir.dt.bfloat16
    AF = mybir.ActivationFunctionType
    ALU = mybir.AluOpType
    AX = mybir.AxisListType

    nc = bass.Bass()

    x0T = nc.declare_dram_parameter("x0T", [DIN, BL], bf16, isOutput=False)
    x1T = nc.declare_dram_parameter("x1T", [DIN, BL], bf16, isOutput=False)
    W0T = nc.declare_dram_parameter("W0T", [DIN, MM], bf16, isOutput=False)
    W1T = nc.declare_dram_parameter("W1T", [DIN, MM], bf16, isOutput=False)
    b0 = nc.declare_dram_parameter("b0", [MM], f32, isOutput=False)
    b1 = nc.declare_dram_parameter("b1", [MM], bf16, isOutput=False)
    WbS = nc.declare_dram_parameter("WbS", [C, S, QT], bf16, isOutput=False)
    bb = nc.declare_dram_parameter("bb", [C, S], bf16, isOutput=False)
    WoT = nc.declare_dram_parameter("WoT", [MM, OUT], bf16, isOutput=False)
    bout = nc.declare_dram_parameter("bout", [OUT], bf16, isOutput=False)
    out = nc.declare_dram_parameter("out", [BL, OUT], f32, isOutput=True)

    h0T_d = nc.dram_tensor("h0T_d", [MM, BL], bf16)
    zb_d = nc.dram_tensor("zb_d", [BL, MM], bf16)
    zn_d = nc.dram_tensor("zn_d", [BL, MM], bf16)

    # middle PSUM fill plan: 13 N-slices (12x512 + 256) packed into 5
    # [128,<=1536] psum tiles (3 banks each, double-buffered = 6 banks)
    FILLS = [(0, (512, 512, 512)), (1536, (512, 512, 512)),
             (3072, (512, 512, 512)), (4608, (512, 512, 512)),
             (6144, (256,))]

    with tile.TileContext(nc) as tc:
        with ExitStack() as top:
            const = top.enter_context(tc.tile_pool(name="const", bufs=1))
            h1p = top.enter_context(tc.tile_pool(name="h1b", bufs=1))
            ps_aux = top.enter_context(
                tc.tile_pool(name="ps_aux", bufs=2, space="PSUM"))

            identb = const.tile([128, 128], bf16)
            identf = const.tile([128, 128], f32)
            make_identity(nc, identf)
            nc.vector.tensor_copy(identb[:], identf[:])

            # b0 as per-partition bias columns: col mt holds m = mt*128 + p
            b0sb = const.tile([128, NMT], f32)
            nc.sync.dma_start(
                out=b0sb[:, : NMT - 1],
                in_=b0[: 128 * (NMT - 1)].rearrange("(j p) -> p j", p=128),
            )
            nc.sync.dma_start(
                out=b0sb[: _msz(NMT - 1), NMT - 1 : NMT],
                in_=b0[128 * (NMT - 1) :].unsqueeze(1),
            )
            b1rep = const.tile([128, MM], bf16)
            nc.sync.dma_start(
                out=b1rep[:], in_=b1[:].unsqueeze(0).broadcast_to([128, MM])
            )
            bbrep = const.tile([128, MM], bf16)
            nc.sync.dma_start(
                out=bbrep[:],
                in_=bb[:].rearrange("c q -> (c q)").unsqueeze(0).broadcast_to(
                    [128, MM]
                ),
            )
            borep = const.tile([128, OUT], bf16)
            nc.sync.dma_start(
                out=borep[:], in_=bout[:].unsqueeze(0).broadcast_to([128, OUT])
            )

            h1b = h1p.tile([128, NBT, MM], bf16)

            # ==== phase 1: A1 (h1b) + A0 (h0T) + middle, fully interleaved ===
            MSL = [(i * 256, 256) for i in range(6)] + [(1536, 64)]
            with ExitStack() as mctx:
                ps_mid = mctx.enter_context(
                    tc.tile_pool(name="ps_mid", bufs=2, space="PSUM"))
                xbp = mctx.enter_context(tc.tile_pool(name="a1x", bufs=1))
                w1p = mctx.enter_context(tc.tile_pool(name="a1w", bufs=2))
                e1p = mctx.enter_context(tc.tile_pool(name="a1e", bufs=2))
                xap = mctx.enter_context(tc.tile_pool(name="a0x", bufs=1))
                w0p = mctx.enter_context(tc.tile_pool(name="a0w", bufs=2))
                e0p = mctx.enter_context(tc.tile_pool(name="a0e", bufs=2))
                wbp = mctx.enter_context(tc.tile_pool(name="wb", bufs=2))
                h0p = mctx.enter_context(tc.tile_pool(name="h0c", bufs=2))
                y2p = mctx.enter_context(tc.tile_pool(name="y2", bufs=2))
                t1p = mctx.enter_context(tc.tile_pool(name="t1", bufs=2))
                t2p = mctx.enter_context(tc.tile_pool(name="t2", bufs=2))
                t3p = mctx.enter_context(tc.tile_pool(name="t3", bufs=2))
                zp = mctx.enter_context(tc.tile_pool(name="zst", bufs=3))

                xb = xbp.tile([128, NKT, BL], bf16)
                nc.sync.dma_start(
                    out=xb[:], in_=x1T[:].rearrange("(k p) b -> p k b", p=128)
                )
                xa = xap.tile([128, NKT, BL], bf16)
                nc.sync.dma_start(
                    out=xa[:], in_=x0T[:].rearrange("(k p) b -> p k b", p=128)
                )

                def emit_a1_ms(msi):
                    mo, mw = MSL[msi]
                    w1 = w1p.tile([128, NKT, 256], bf16, tag="w1")
                    nc.sync.dma_start(
                        out=w1[:, :, :mw],
                        in_=W1T[:, mo : mo + mw].rearrange(
                            "(k p) m -> p k m", p=128
                        ),
                    )
                    for bt in range(NBT):
                        ps = ps_aux.tile([128, 512], f32, tag="psa")
                        for k in range(NKT):
                            nc.tensor.matmul(
                                ps[:, :mw],
                                lhsT=xb[:, k, bt * 128 : (bt + 1) * 128],
                                rhs=w1[:, k, :mw],
                                start=(k == 0),
                                stop=(k == NKT - 1),
                            )
                        ev = e1p.tile([128, 256], bf16, tag="e1")
                        nc.scalar.activation(ev[:, :mw], ps[:, :mw], AF.Identity)
                        nc.vector.tensor_tensor(
                            out=h1b[:, bt, mo : mo + mw],
                            in0=ev[:, :mw],
                            in1=b1rep[:, mo : mo + mw],
                            op=ALU.add,
                        )

                def emit_a0_mt(mt):
                    ms = _msz(mt)
                    msl = slice(mt * 128, mt * 128 + ms)
                    w0 = w0p.tile([128, NKT, 128], bf16, tag="w0")
                    nc.sync.dma_start(
                        out=w0[:, :, :ms],
                        in_=W0T[:, msl].rearrange("(k p) m -> p k m", p=128),
                    )
                    for hh in range(2):
                        hsl = slice(hh * 512, (hh + 1) * 512)
                        ps = ps_aux.tile([128, 512], f32, tag="psa")
                        for k in range(NKT):
                            nc.tensor.matmul(
                                ps[:ms, :],
                                lhsT=w0[:, k, :ms],
                                rhs=xa[:, k, hsl],
                                start=(k == 0),
                                stop=(k == NKT - 1),
                            )
                        ev = e0p.tile([128, 512], bf16, tag="e0")
                        nc.scalar.activation(
                            ev[:ms, :], ps[:ms, :], AF.Identity,
                            bias=b0sb[:ms, mt : mt + 1],
                        )
                        nc.sync.dma_start(out=h0T_d[msl, hsl], in_=ev[:ms, :])

                def emit_chunk(c):
                    csl = slice(c * S, (c + 1) * S)
                    wbs = wbp.tile([S, QT], bf16, tag="wbs")
                    nc.sync.dma_start(out=wbs[:], in_=WbS[c])
                    h0c = h0p.tile([S, BL], bf16, tag="h0c")
                    nc.sync.dma_start(out=h0c[:], in_=h0T_d[csl, :])
                    for bt in range(NBT):
                        bsl = slice(bt * 128, (bt + 1) * 128)
                        y2 = y2p.tile([128, QT], bf16, tag="y2")
                        for off, ws in FILLS:
                            fw = sum(ws)
                            ps = ps_mid.tile([128, 1536], f32, tag="mid")
                            o = 0
                            for w in ws:
                                nc.tensor.matmul(
                                    ps[:, o : o + w],
                                    lhsT=h0c[:, bsl],
                                    rhs=wbs[:, off + o : off + o + w],
                                    start=True,
                                    stop=True,
                                )
                                o += w
                            nc.scalar.activation(
                                y2[:, off : off + fw], ps[:, :fw], AF.Identity
                            )
                        y23 = y2[:].rearrange("p (q t) -> p q t", t=S)
                        nc.vector.tensor_tensor(
                            out=y23,
                            in0=y23,
                            in1=h1b[:, bt, csl].unsqueeze(1).broadcast_to(
                                [128, S, S]
                            ),
                            op=ALU.mult,
                        )
                        t1 = t1p.tile([128, S, 40], bf16, tag="t1")
                        nc.vector.tensor_tensor(
                            out=t1[:], in0=y23[:, :, :40],
                            in1=y23[:, :, 40:], op=ALU.add,
                        )
                        t2 = t2p.tile([128, S, 20], bf16, tag="t2")
                        nc.gpsimd.tensor_tensor(
                            out=t2[:], in0=t1[:, :, :20], in1=t1[:, :, 20:],
                            op=ALU.add,
                        )
                        t3 = t3p.tile([128, S, 10], bf16, tag="t3")
                        nc.gpsimd.tensor_tensor(
                            out=t3[:], in0=t2[:, :, :10], in1=t2[:, :, 10:],
                            op=ALU.add,
                        )
                        zst = zp.tile([128, S], bf16, tag="zst")
                        with nc.allow_low_precision(
                            reason="80-term sum accumulates fp32 in-engine"
                        ):
                            nc.vector.tensor_reduce(
                                out=zst[:], in_=t3[:], axis=AX.X, op=ALU.add
                            )
                        nc.sync.dma_start(out=zb_d[bsl, csl], in_=zst[:])

                # chunk c is emittable once A0 covers rows 80c+79 (mt) and A1
                # covers cols 80c+79 (ms)
                def ms_hi(c):
                    return min(len(MSL) - 1, (80 * c + 79) // 256)

                def mt_hi(c):
                    return (80 * c + 79) // 128

                done = 0
                ms_done = 0
                emit_a1_ms(0)
                ms_done = 1
                for mt in range(NMT):
                    emit_a0_mt(mt)
                    ready = min(C, (128 * (mt + 1)) // 80)
                    while done < ready:
                        if ms_hi(done) >= ms_done:
                            emit_a1_ms(ms_done)
                            ms_done += 1
                        emit_chunk(done)
                        done += 1
                while ms_done < len(MSL):
                    emit_a1_ms(ms_done)
                    ms_done += 1
                while done < C:
                    emit_chunk(done)
                    done += 1

            # ==== phase 2: tail + znT + out-proj, per batch-tile ====
            with ExitStack() as octx:
                ps_out = octx.enter_context(
                    tc.tile_pool(name="ps_out", bufs=2, space="PSUM"))
                zlp = octx.enter_context(tc.tile_pool(name="tl", bufs=2))
                znpp = octx.enter_context(tc.tile_pool(name="zn", bufs=2))
                sp = octx.enter_context(tc.tile_pool(name="tls", bufs=2))
                znp = octx.enter_context(tc.tile_pool(name="znT", bufs=1))
                wop = octx.enter_context(tc.tile_pool(name="wo", bufs=1))
                evp = octx.enter_context(tc.tile_pool(name="oev", bufs=3))

                # all 6 Wout column groups resident (bf16, 80KB)
                wos = wop.tile([128, NOG, NMT, 512], bf16)
                for og in range(NOG):
                    ow = _osz(og)
                    osl = slice(og * 512, og * 512 + ow)
                    nc.sync.dma_start(
                        out=wos[:, og, : NMT - 1, :ow],
                        in_=WoT[: 128 * (NMT - 1), osl].rearrange(
                            "(k p) o -> p k o", p=128
                        ),
                    )
                    nc.sync.dma_start(
                        out=wos[: _msz(NMT - 1), og, NMT - 1, :ow],
                        in_=WoT[128 * (NMT - 1) :, osl],
                    )

                znT = znp.tile([128, NMT, 128], bf16)  # one bt at a time
                for bt in range(NBT):
                    bsl = slice(bt * 128, (bt + 1) * 128)
                    zt = zlp.tile([128, MM], bf16, tag="zt")
                    nc.sync.dma_start(out=zt[:], in_=zb_d[bsl, :])
                    zbb = zlp.tile([128, MM], bf16, tag="zbb")
                    nc.vector.tensor_tensor(
                        out=zbb[:], in0=zt[:], in1=bbrep[:], op=ALU.add
                    )
                    sab = zlp.tile([128, MM], bf16, tag="zt")
                    nc.scalar.activation(sab[:], zbb[:], AF.Abs)
                    sq = zlp.tile([128, MM], bf16, tag="sq")
                    nc.scalar.activation(sq[:], sab[:], AF.Sqrt)
                    sgn = zlp.tile([128, MM], bf16, tag="sgn")
                    nc.scalar.activation(sgn[:], zbb[:], AF.Sign)
                    ss = zlp.tile([128, MM], bf16, tag="ss")
                    nc.vector.tensor_tensor(
                        out=ss[:], in0=sgn[:], in1=sq[:], op=ALU.mult
                    )
                    nsq = sp.tile([128, C], f32, tag="nsq")
                    nc.vector.tensor_reduce(
                        out=nsq[:],
                        in_=zbb[:].rearrange("p (c q) -> p c q", q=S),
                        axis=AX.X,
                        op=ALU.add,
                        apply_absolute_value=True,
                    )
                    nrm = sp.tile([128, C], f32, tag="nrm")
                    nc.scalar.activation(nrm[:], nsq[:], AF.Sqrt)
                    nrmc = sp.tile([128, C], f32, tag="nrmc")
                    nc.vector.tensor_scalar_max(
                        out=nrmc[:], in0=nrm[:], scalar1=EPS
                    )
                    inv = sp.tile([128, C], f32, tag="inv")
                    nc.vector.reciprocal(inv[:], nrmc[:])
                    zn = znpp.tile([128, MM], bf16, tag="zn")
                    nc.vector.tensor_tensor(
                        out=zn[:].rearrange("p (c q) -> p c q", q=S),
                        in0=ss[:].rearrange("p (c q) -> p c q", q=S),
                        in1=inv[:].unsqueeze(2).broadcast_to([128, C, S]),
                        op=ALU.mult,
                    )
                    # transpose this bt into znT, then its 6 out-proj groups
                    for kq in range(4):
                        pst = ps_out.tile([128, 512], bf16, tag="pst")
                        nw = min(4, NMT - kq * 4)
                        for i in range(nw):
                            k = kq * 4 + i
                            ks = _msz(k)
                            nc.tensor.transpose(
                                pst[:ks, i * 128 : i * 128 + 128],
                                zn[:, k * 128 : k * 128 + ks],
                                identb[:],
                            )
                        nc.scalar.activation(
                            znT[:, kq * 4 : kq * 4 + nw, :],
                            pst[:, : nw * 128].rearrange(
                                "p (k b) -> p k b", b=128
                            ),
                            AF.Identity,
                        )
                    for og in range(NOG):
                        ow = _osz(og)
                        osl = slice(og * 512, og * 512 + ow)
                        ps = ps_out.tile([128, 512], f32, tag="acc")
                        for k in range(NMT):
                            ks = _msz(k)
                            nc.tensor.matmul(
                                ps[:, :ow],
                                lhsT=znT[:ks, k, :],
                                rhs=wos[:ks, og, k, :ow],
                                start=(k == 0),
                                stop=(k == NMT - 1),
                            )
                        ev = evp.tile([128, 512], f32, tag="oev")
                        nc.vector.tensor_tensor(
                            out=ev[:, :ow], in0=ps[:, :ow], in1=borep[:, osl],
                            op=ALU.add,
                        )
                        nc.sync.dma_start(out=out[bsl, osl], in_=ev[:, :ow])

    _split_excess_waits(nc, cap=4)
    return nc


def _split_excess_waits(nc, cap=4):
    """Walrus rejects instructions with too many sync waits. Move excess
    waits onto NoOps spliced just before the instruction on the same engine
    queue (the sequencer executes them in order, so semantics are identical).
    """
    import concourse.mybir as mybir
    import bass_rust

    n = 0
    for f in nc.m.functions:
        for blk in f.blocks:
            out = []
            changed = False
            for inst in blk.instructions:
                si = getattr(inst, "sync_info", None)
                waits = list(si.on_wait) if si is not None and si.on_wait else []
                icap = 2 if inst.opcode == "EventSemaphore" else 1
                if len(waits) > icap:
                    excess, keep = waits[:-icap], waits[-icap:]
                    for w in excess:
                        nop = mybir.InstNoOp(
                            name=f"{inst.name}-wsplit{n}", ins=[], outs=[]
                        )
                        n += 1
                        nop.engine = inst.engine
                        nop.sync_info = bass_rust.SyncInfo(
                            on_wait=[w], on_update=[]
                        )
                        out.append(nop)
                    inst.sync_info = bass_rust.SyncInfo(
                        on_wait=keep, on_update=list(si.on_update or [])
                    )
                    changed = True
                out.append(inst)
            if changed:
                blk.instructions = out
    return nc


def _get_nc():
    if "nc" not in _CACHE:
        _CACHE["nc"] = _build()
    return _CACHE["nc"]


def _shard_inputs(inputs):
    import ml_dtypes

    bf = ml_dtypes.bfloat16
    full = {k: np.asarray(v, dtype=np.float32) for k, v in inputs.items()}
    shared = {
        "W0T": np.ascontiguousarray(full["W0"].T.astype(bf)),
        "W1T": np.ascontiguousarray(full["W1"].T.astype(bf)),
        "WbS": np.ascontiguousarray(
            full["Wb"].transpose(0, 2, 1, 3).reshape(C, S, QT).astype(bf)
        ),
        "WoT": np.ascontiguousarray(full["Wout"].T.astype(bf)),
        "b0": np.ascontiguousarray(full["b0"]),
        "b1": np.ascontiguousarray(full["b1"].astype(bf)),
        "bb": np.ascontiguousarray(full["bb"].astype(bf)),
        "bout": np.ascontiguousarray(full["bout"].astype(bf)),
    }
    rows = full["x0"].shape[0] // NCORES
    in_maps = []
    for i in range(NCORES):
        m = dict(shared)
        m["x0T"] = np.ascontiguousarray(
            full["x0"][i * rows : (i + 1) * rows].T.astype(bf)
        )
        m["x1T"] = np.ascontiguousarray(
            full["x1"][i * rows : (i + 1) * rows].T.astype(bf)
        )
        in_maps.append(m)
    return in_maps


def kernel(**inputs):
    from concourse.bass_utils import run_bass_kernel_spmd

    nc = _get_nc()
    in_maps = _shard_inputs(inputs)
    res = run_bass_kernel_spmd(nc, in_maps, list(range(NCORES)))
    return np.concatenate([res.results[i]["out"] for i in range(NCORES)], axis=0)


# revision 23
# speedup vs baseline: 1.0428x; 1.0220x over previous
"""BlockTucker kernel for TRN2, 8 NeuronCores, data-parallel over batch.

Model (per reference):
    h0 = (x0 @ W0.T + b0).reshape(B, C, S)          B=8192 DIN=2048 MM=1600
    h1 = (x1 @ W1.T + b1).reshape(B, C, S)          C=20 chunks, S=80
    z[b,c,q] = sum_{s,t} h0[b,c,s] Wb[c,q,s,t] h1[b,c,t] + bb[c,q]
    z = signed_sqrt(z); z = z / max(||z||_chunk, eps); out = z @ Wout.T + bout

v2 dataflow (BL = 1024 rows/core; weights host-pretransposed + bf16):
  stage A0 (PE): h0T[m, b] = W0T.T @ x0T + b0  -> DRAM bf16 (lhsT for middle)
  stage A1 (PE): h1b[b, m] = x1T.T @ W1T       -> SBUF bf16 (+b1 via DVE)
  middle, per chunk c / batch-tile bt (q-major free dim):
      PE:   Y2[b,(q,t)] = h0c[s,b].T @ WbS[c][s,(q,t)]   (K=80, bf16)
      ACT:  evacuate PSUM -> y2 bf16 (5 slices, 3-bank-wide)
      DVE:  gate g = y2 * h1b bcast (2x bf16); tree t1 (+t2 on parity)
      Pool: tree t3 (+t2 on parity) + final reduce -> zst bf16 -> zb DRAM
  tail (DVE/ACT/Pool): +bb, signed-sqrt via rsqrt, per-chunk L2 norm -> zn
  out-proj (PE): znT on-chip transposes; out = znT.T @ WoutT + bout
"""

import numpy as np

BL = 1024
DIN = 2048
MM = 1600
C, S = 20, 80
OUT = 3000
NCORES = 8
QT = S * S
NKT = DIN // 128            # 16
NMT = 13                    # 12x128 + 64
NBT = BL // 128             # 8
NOG = 6                     # out column groups: 5x512 + 440
EPS = 1e-12

_CACHE = {}


def _msz(mt):
    return 128 if mt < NMT - 1 else MM - 128 * (NMT - 1)


def _osz(og):
    return 512 if og < NOG - 1 else OUT - 512 * (NOG - 1)


def _build():
    import concourse.bass as bass
    import concourse.mybir as mybir
    import concourse.tile as tile
    from concourse.masks import make_identity
    from contextlib import ExitStack

    f32 = mybir.dt.float32
    bf16 = mybir.dt.bfloat16
    AF = mybir.ActivationFunctionType
    ALU = mybir.AluOpType
    AX = mybir.AxisListType

    nc = bass.Bass()

    x0T = nc.declare_dram_parameter("x0T", [DIN, BL], bf16, isOutput=False)
    x1T = nc.declare_dram_parameter("x1T", [DIN, BL], bf16, isOutput=False)
    W0T = nc.declare_dram_parameter("W0T", [DIN, MM], bf16, isOutput=False)
    W1T = nc.declare_dram_parameter("W1T", [DIN, MM], bf16, isOutput=False)
    b0 = nc.declare_dram_parameter("b0", [MM], f32, isOutput=False)
    b1 = nc.declare_dram_parameter("b1", [MM], bf16, isOutput=False)
    WbS = nc.declare_dram_parameter("WbS", [C, S, QT], bf16, isOutput=False)
    bb = nc.declare_dram_parameter("bb", [C, S], bf16, isOutput=False)
    WoT = nc.declare_dram_parameter("WoT", [MM, OUT], bf16, isOutput=False)
    bout = nc.declare_dram_parameter("bout", [OUT], bf16, isOutput=False)
    out = nc.declare_dram_parameter("out", [BL, OUT], f32, isOutput=True)

    h0T_d = nc.dram_tensor("h0T_d", [MM, BL], bf16)
    zb_d = nc.dram_tensor("zb_d", [BL, MM], bf16)
    zn_d = nc.dram_tensor("zn_d", [BL, MM], bf16)

    # middle PSUM fill plan: 13 N-slices (12x512 + 256) packed into 5
    # [128,<=1536] psum tiles (3 banks each, double-buffered = 6 banks)
    FILLS = [(0, (512, 512, 512)), (1536, (512, 512, 512)),
             (3072, (512, 512, 512)), (4608, (512, 512, 512)),
             (6144, (256,))]

    with tile.TileContext(nc) as tc:
        with ExitStack() as top:
            const = top.enter_context(tc.tile_pool(name="const", bufs=1))
            h1p = top.enter_context(tc.tile_pool(name="h1b", bufs=1))
            ps_aux = top.enter_context(
                tc.tile_pool(name="ps_aux", bufs=2, space="PSUM"))

            identb = const.tile([128, 128], bf16)
            identf = const.tile([128, 128], f32)
            make_identity(nc, identf)
            nc.vector.tensor_copy(identb[:], identf[:])

            # b0 as per-partition bias columns: col mt holds m = mt*128 + p
            b0sb = const.tile([128, NMT], f32)
            nc.sync.dma_start(
                out=b0sb[:, : NMT - 1],
                in_=b0[: 128 * (NMT - 1)].rearrange("(j p) -> p j", p=128),
            )
            nc.sync.dma_start(
                out=b0sb[: _msz(NMT - 1), NMT - 1 : NMT],
                in_=b0[128 * (NMT - 1) :].unsqueeze(1),
            )
            b1rep = const.tile([128, MM], bf16)
            nc.sync.dma_start(
                out=b1rep[:], in_=b1[:].unsqueeze(0).broadcast_to([128, MM])
            )
            bbrep = const.tile([128, MM], bf16)
            nc.sync.dma_start(
                out=bbrep[:],
                in_=bb[:].rearrange("c q -> (c q)").unsqueeze(0).broadcast_to(
                    [128, MM]
                ),
            )
            borep = const.tile([128, OUT], bf16)
            nc.sync.dma_start(
                out=borep[:], in_=bout[:].unsqueeze(0).broadcast_to([128, OUT])
            )

            h1b = h1p.tile([128, NBT, MM], bf16)

            # ==== phase 1: A1 (h1b) + A0 (h0T) + middle, fully interleaved ===
            MSL = [(i * 256, 256) for i in range(6)] + [(1536, 64)]
            with ExitStack() as mctx:
                ps_mid = mctx.enter_context(
                    tc.tile_pool(name="ps_mid", bufs=2, space="PSUM"))
                xbp = mctx.enter_context(tc.tile_pool(name="a1x", bufs=1))
                w1p = mctx.enter_context(tc.tile_pool(name="a1w", bufs=2))
                e1p = mctx.enter_context(tc.tile_pool(name="a1e", bufs=2))
                xap = mctx.enter_context(tc.tile_pool(name="a0x", bufs=1))
                w0p = mctx.enter_context(tc.tile_pool(name="a0w", bufs=2))
                e0p = mctx.enter_context(tc.tile_pool(name="a0e", bufs=2))
                wbp = mctx.enter_context(tc.tile_pool(name="wb", bufs=2))
                h0p = mctx.enter_context(tc.tile_pool(name="h0c", bufs=2))
                y2p = mctx.enter_context(tc.tile_pool(name="y2", bufs=2))
                t1p = mctx.enter_context(tc.tile_pool(name="t1", bufs=2))
                t2p = mctx.enter_context(tc.tile_pool(name="t2", bufs=2))
                t3p = mctx.enter_context(tc.tile_pool(name="t3", bufs=3))
                zp = mctx.enter_context(tc.tile_pool(name="zst", bufs=3))

                pending = []
                xb = xbp.tile([128, NKT, BL], bf16)
                nc.sync.dma_start(
                    out=xb[:], in_=x1T[:].rearrange("(k p) b -> p k b", p=128)
                )
                xa = xap.tile([128, NKT, BL], bf16)
                nc.sync.dma_start(
                    out=xa[:], in_=x0T[:].rearrange("(k p) b -> p k b", p=128)
                )

                def emit_a1_ms(msi):
                    mo, mw = MSL[msi]
                    w1 = w1p.tile([128, NKT, 256], bf16, tag="w1")
                    nc.sync.dma_start(
                        out=w1[:, :, :mw],
                        in_=W1T[:, mo : mo + mw].rearrange(
                            "(k p) m -> p k m", p=128
                        ),
                    )
                    for bt in range(NBT):
                        ps = ps_aux.tile([128, 512], f32, tag="psa")
                        for k in range(NKT):
                            nc.tensor.matmul(
                                ps[:, :mw],
                                lhsT=xb[:, k, bt * 128 : (bt + 1) * 128],
                                rhs=w1[:, k, :mw],
                                start=(k == 0),
                                stop=(k == NKT - 1),
                            )
                        ev = e1p.tile([128, 256], bf16, tag="e1")
                        nc.scalar.activation(ev[:, :mw], ps[:, :mw], AF.Identity)
                        nc.vector.tensor_tensor(
                            out=h1b[:, bt, mo : mo + mw],
                            in0=ev[:, :mw],
                            in1=b1rep[:, mo : mo + mw],
                            op=ALU.add,
                        )

                def emit_a0_mt(mt):
                    ms = _msz(mt)
                    msl = slice(mt * 128, mt * 128 + ms)
                    w0 = w0p.tile([128, NKT, 128], bf16, tag="w0")
                    nc.sync.dma_start(
                        out=w0[:, :, :ms],
                        in_=W0T[:, msl].rearrange("(k p) m -> p k m", p=128),
                    )
                    for hh in range(2):
                        hsl = slice(hh * 512, (hh + 1) * 512)
                        ps = ps_aux.tile([128, 512], f32, tag="psa")
                        for k in range(NKT):
                            nc.tensor.matmul(
                                ps[:ms, :],
                                lhsT=w0[:, k, :ms],
                                rhs=xa[:, k, hsl],
                                start=(k == 0),
                                stop=(k == NKT - 1),
                            )
                        ev = e0p.tile([128, 512], bf16, tag="e0")
                        nc.scalar.activation(
                            ev[:ms, :], ps[:ms, :], AF.Identity,
                            bias=b0sb[:ms, mt : mt + 1],
                        )
                        nc.sync.dma_start(out=h0T_d[msl, hsl], in_=ev[:ms, :])

                def emit_chunk(c):
                    csl = slice(c * S, (c + 1) * S)
                    wbs = wbp.tile([S, QT], bf16, tag="wbs")
                    nc.sync.dma_start(out=wbs[:], in_=WbS[c])
                    h0c = h0p.tile([S, BL], bf16, tag="h0c")
                    nc.sync.dma_start(out=h0c[:], in_=h0T_d[csl, :])
                    for bt in range(NBT):
                        bsl = slice(bt * 128, (bt + 1) * 128)
                        if len(pending) > 1:
                            pending.pop(0)()
                        y2 = y2p.tile([128, QT], bf16, tag="y2")
                        for off, ws in FILLS:
                            fw = sum(ws)
                            ps = ps_mid.tile([128, 1536], f32, tag="mid")
                            o = 0
                            for w in ws:
                                nc.tensor.matmul(
                                    ps[:, o : o + w],
                                    lhsT=h0c[:, bsl],
                                    rhs=wbs[:, off + o : off + o + w],
                                    start=True,
                                    stop=True,
                                )
                                o += w
                            nc.scalar.activation(
                                y2[:, off : off + fw], ps[:, :fw], AF.Identity
                            )
                        y23 = y2[:].rearrange("p (q t) -> p q t", t=S)
                        nc.vector.tensor_tensor(
                            out=y23,
                            in0=y23,
                            in1=h1b[:, bt, csl].unsqueeze(1).broadcast_to(
                                [128, S, S]
                            ),
                            op=ALU.mult,
                        )
                        t1 = t1p.tile([128, S, 40], bf16, tag="t1")
                        nc.vector.tensor_tensor(
                            out=t1[:], in0=y23[:, :, :40],
                            in1=y23[:, :, 40:], op=ALU.add,
                        )
                        t2 = t2p.tile([128, S, 20], bf16, tag="t2")
                        nc.gpsimd.tensor_tensor(
                            out=t2[:], in0=t1[:, :, :20], in1=t1[:, :, 20:],
                            op=ALU.add,
                        )
                        t3 = t3p.tile([128, S, 10], bf16, tag="t3")
                        nc.gpsimd.tensor_tensor(
                            out=t3[:], in0=t2[:, :, :10], in1=t2[:, :, 10:],
                            op=ALU.add,
                        )

                        def fin(t3=t3, bsl=bsl, csl=csl):
                            zst = zp.tile([128, S], bf16, tag="zst")
                            with nc.allow_low_precision(
                                reason="80-term sum accumulates fp32 in-engine"
                            ):
                                nc.vector.tensor_reduce(
                                    out=zst[:], in_=t3[:], axis=AX.X, op=ALU.add
                                )
                            nc.sync.dma_start(out=zb_d[bsl, csl], in_=zst[:])

                        pending.append(fin)

                # chunk c is emittable once A0 covers rows 80c+79 (mt) and A1
                # covers cols 80c+79 (ms)
                def ms_hi(c):
                    return min(len(MSL) - 1, (80 * c + 79) // 256)

                def mt_hi(c):
                    return (80 * c + 79) // 128

                done = 0
                ms_done = 0
                emit_a1_ms(0)
                ms_done = 1
                for mt in range(NMT):
                    emit_a0_mt(mt)
                    ready = min(C, (128 * (mt + 1)) // 80)
                    while done < ready:
                        if ms_hi(done) >= ms_done:
                            emit_a1_ms(ms_done)
                            ms_done += 1
                        emit_chunk(done)
                        done += 1
                while ms_done < len(MSL):
                    emit_a1_ms(ms_done)
                    ms_done += 1
                while done < C:
                    emit_chunk(done)
                    done += 1
                for fin in pending:
                    fin()
                pending.clear()

            # ==== phase 2: tail + znT + out-proj, per batch-tile ====
            with ExitStack() as octx:
                ps_out = octx.enter_context(
                    tc.tile_pool(name="ps_out", bufs=2, space="PSUM"))
                zlp = octx.enter_context(tc.tile_pool(name="tl", bufs=2))
                znpp = octx.enter_context(tc.tile_pool(name="zn", bufs=2))
                sp = octx.enter_context(tc.tile_pool(name="tls", bufs=2))
                znp = octx.enter_context(tc.tile_pool(name="znT", bufs=1))
                wop = octx.enter_context(tc.tile_pool(name="wo", bufs=1))
                evp = octx.enter_context(tc.tile_pool(name="oev", bufs=3))

                # all 6 Wout column groups resident (bf16, 80KB)
                wos = wop.tile([128, NOG, NMT, 512], bf16)
                for og in range(NOG):
                    ow = _osz(og)
                    osl = slice(og * 512, og * 512 + ow)
                    nc.sync.dma_start(
                        out=wos[:, og, : NMT - 1, :ow],
                        in_=WoT[: 128 * (NMT - 1), osl].rearrange(
                            "(k p) o -> p k o", p=128
                        ),
                    )
                    nc.sync.dma_start(
                        out=wos[: _msz(NMT - 1), og, NMT - 1, :ow],
                        in_=WoT[128 * (NMT - 1) :, osl],
                    )

                znT = znp.tile([128, NMT, 128], bf16)  # one bt at a time
                for bt in range(NBT):
                    bsl = slice(bt * 128, (bt + 1) * 128)
                    zt = zlp.tile([128, MM], bf16, tag="zt")
                    nc.sync.dma_start(out=zt[:], in_=zb_d[bsl, :])
                    zbb = zlp.tile([128, MM], bf16, tag="zbb")
                    nc.vector.tensor_tensor(
                        out=zbb[:], in0=zt[:], in1=bbrep[:], op=ALU.add
                    )
                    sab = zlp.tile([128, MM], bf16, tag="zt")
                    nc.scalar.activation(sab[:], zbb[:], AF.Abs)
                    sq = zlp.tile([128, MM], bf16, tag="sq")
                    nc.scalar.activation(sq[:], sab[:], AF.Sqrt)
                    sgn = zlp.tile([128, MM], bf16, tag="sgn")
                    nc.scalar.activation(sgn[:], zbb[:], AF.Sign)
                    ss = zlp.tile([128, MM], bf16, tag="ss")
                    nc.vector.tensor_tensor(
                        out=ss[:], in0=sgn[:], in1=sq[:], op=ALU.mult
                    )
                    nsq = sp.tile([128, C], f32, tag="nsq")
                    nc.vector.tensor_reduce(
                        out=nsq[:],
                        in_=zbb[:].rearrange("p (c q) -> p c q", q=S),
                        axis=AX.X,
                        op=ALU.add,
                        apply_absolute_value=True,
                    )
                    nrm = sp.tile([128, C], f32, tag="nrm")
                    nc.scalar.activation(nrm[:], nsq[:], AF.Sqrt)
                    nrmc = sp.tile([128, C], f32, tag="nrmc")
                    nc.vector.tensor_scalar_max(
                        out=nrmc[:], in0=nrm[:], scalar1=EPS
                    )
                    inv = sp.tile([128, C], f32, tag="inv")
                    nc.vector.reciprocal(inv[:], nrmc[:])
                    zn = znpp.tile([128, MM], bf16, tag="zn")
                    nc.vector.tensor_tensor(
                        out=zn[:].rearrange("p (c q) -> p c q", q=S),
                        in0=ss[:].rearrange("p (c q) -> p c q", q=S),
                        in1=inv[:].unsqueeze(2).broadcast_to([128, C, S]),
                        op=ALU.mult,
                    )
                    # transpose this bt into znT, then its 6 out-proj groups
                    for kq in range(4):
                        pst = ps_out.tile([128, 512], bf16, tag="pst")
                        nw = min(4, NMT - kq * 4)
                        for i in range(nw):
                            k = kq * 4 + i
                            ks = _msz(k)
                            nc.tensor.transpose(
                                pst[:ks, i * 128 : i * 128 + 128],
                                zn[:, k * 128 : k * 128 + ks],
                                identb[:],
                            )
                        nc.scalar.activation(
                            znT[:, kq * 4 : kq * 4 + nw, :],
                            pst[:, : nw * 128].rearrange(
                                "p (k b) -> p k b", b=128
                            ),
                            AF.Identity,
                        )
                    for og in range(NOG):
                        ow = _osz(og)
                        osl = slice(og * 512, og * 512 + ow)
                        ps = ps_out.tile([128, 512], f32, tag="acc")
                        for k in range(NMT):
                            ks = _msz(k)
                            nc.tensor.matmul(
                                ps[:, :ow],
                                lhsT=znT[:ks, k, :],
                                rhs=wos[:ks, og, k, :ow],
                                start=(k == 0),
                                stop=(k == NMT - 1),
                            )
                        ev = evp.tile([128, 512], f32, tag="oev")
                        nc.vector.tensor_tensor(
                            out=ev[:, :ow], in0=ps[:, :ow], in1=borep[:, osl],
                            op=ALU.add,
                        )
                        nc.sync.dma_start(out=out[bsl, osl], in_=ev[:, :ow])

    _split_excess_waits(nc, cap=4)
    return nc


def _split_excess_waits(nc, cap=4):
    """Walrus rejects instructions with too many sync waits. Move excess
    waits onto NoOps spliced just before the instruction on the same engine
    queue (the sequencer executes them in order, so semantics are identical).
    """
    import concourse.mybir as mybir
    import bass_rust

    n = 0
    for f in nc.m.functions:
        for blk in f.blocks:
            out = []
            changed = False
            for inst in blk.instructions:
                si = getattr(inst, "sync_info", None)
                waits = list(si.on_wait) if si is not None and si.on_wait else []
                icap = 2 if inst.opcode == "EventSemaphore" else 1
                if len(waits) > icap:
                    excess, keep = waits[:-icap], waits[-icap:]
                    for w in excess:
                        nop = mybir.InstNoOp(
                            name=f"{inst.name}-wsplit{n}", ins=[], outs=[]
                        )
                        n += 1
                        nop.engine = inst.engine
                        nop.sync_info = bass_rust.SyncInfo(
                            on_wait=[w], on_update=[]
                        )
                        out.append(nop)
                    inst.sync_info = bass_rust.SyncInfo(
                        on_wait=keep, on_update=list(si.on_update or [])
                    )
                    changed = True
                out.append(inst)
            if changed:
                blk.instructions = out
    return nc


def _get_nc():
    if "nc" not in _CACHE:
        _CACHE["nc"] = _build()
    return _CACHE["nc"]


def _shard_inputs(inputs):
    import ml_dtypes

    bf = ml_dtypes.bfloat16
    full = {k: np.asarray(v, dtype=np.float32) for k, v in inputs.items()}
    shared = {
        "W0T": np.ascontiguousarray(full["W0"].T.astype(bf)),
        "W1T": np.ascontiguousarray(full["W1"].T.astype(bf)),
        "WbS": np.ascontiguousarray(
            full["Wb"].transpose(0, 2, 1, 3).reshape(C, S, QT).astype(bf)
        ),
        "WoT": np.ascontiguousarray(full["Wout"].T.astype(bf)),
        "b0": np.ascontiguousarray(full["b0"]),
        "b1": np.ascontiguousarray(full["b1"].astype(bf)),
        "bb": np.ascontiguousarray(full["bb"].astype(bf)),
        "bout": np.ascontiguousarray(full["bout"].astype(bf)),
    }
    rows = full["x0"].shape[0] // NCORES
    in_maps = []
    for i in range(NCORES):
        m = dict(shared)
        m["x0T"] = np.ascontiguousarray(
            full["x0"][i * rows : (i + 1) * rows].T.astype(bf)
        )
        m["x1T"] = np.ascontiguousarray(
            full["x1"][i * rows : (i + 1) * rows].T.astype(bf)
        )
        in_maps.append(m)
    return in_maps


def kernel(**inputs):
    from concourse.bass_utils import run_bass_kernel_spmd

    nc = _get_nc()
    in_maps = _shard_inputs(inputs)
    res = run_bass_kernel_spmd(nc, in_maps, list(range(NCORES)))
    return np.concatenate([res.results[i]["out"] for i in range(NCORES)], axis=0)


# revision 27
# speedup vs baseline: 1.2261x; 1.1758x over previous
"""BlockTucker kernel for TRN2, 8 NeuronCores, data-parallel over batch.

Model (per reference):
    h0 = (x0 @ W0.T + b0).reshape(B, C, S)          B=8192 DIN=2048 MM=1600
    h1 = (x1 @ W1.T + b1).reshape(B, C, S)          C=20 chunks, S=80
    z[b,c,q] = sum_{s,t} h0[b,c,s] Wb[c,q,s,t] h1[b,c,t] + bb[c,q]
    z = signed_sqrt(z); z = z / max(||z||_chunk, eps); out = z @ Wout.T + bout

v2 dataflow (BL = 1024 rows/core; weights host-pretransposed + bf16):
  stage A0 (PE): h0T[m, b] = W0T.T @ x0T + b0  -> DRAM bf16 (lhsT for middle)
  stage A1 (PE): h1b[b, m] = x1T.T @ W1T       -> SBUF bf16 (+b1 via DVE)
  middle, per chunk c / batch-tile bt (q-major free dim):
      PE:   Y2[b,(q,t)] = h0c[s,b].T @ WbS[c][s,(q,t)]   (K=80, bf16)
      ACT:  evacuate PSUM -> y2 bf16 (5 slices, 3-bank-wide)
      DVE:  gate g = y2 * h1b bcast (2x bf16); tree t1 (+t2 on parity)
      Pool: tree t3 (+t2 on parity) + final reduce -> zst bf16 -> zb DRAM
  tail (DVE/ACT/Pool): +bb, signed-sqrt via rsqrt, per-chunk L2 norm -> zn
  out-proj (PE): znT on-chip transposes; out = znT.T @ WoutT + bout
"""

import numpy as np

BL = 1024
DIN = 2048
MM = 1600
C, S = 20, 80
OUT = 3000
NCORES = 8
QT = S * S
NKT = DIN // 128            # 16
NMT = 13                    # 12x128 + 64
NBT = BL // 128             # 8
NOG = 6                     # out column groups: 5x512 + 440
EPS = 1e-12

_CACHE = {}


def _msz(mt):
    return 128 if mt < NMT - 1 else MM - 128 * (NMT - 1)


def _osz(og):
    return 512 if og < NOG - 1 else OUT - 512 * (NOG - 1)


def _build():
    import concourse.bass as bass
    import concourse.mybir as mybir
    import concourse.tile as tile
    from concourse.masks import make_identity
    from contextlib import ExitStack

    f32 = mybir.dt.float32
    bf16 = mybir.dt.bfloat16
    AF = mybir.ActivationFunctionType
    ALU = mybir.AluOpType
    AX = mybir.AxisListType

    nc = bass.Bass()

    x0T = nc.declare_dram_parameter("x0T", [DIN, BL], bf16, isOutput=False)
    x1T = nc.declare_dram_parameter("x1T", [DIN, BL], bf16, isOutput=False)
    W0T = nc.declare_dram_parameter("W0T", [DIN, MM], bf16, isOutput=False)
    W1T = nc.declare_dram_parameter("W1T", [DIN, MM], bf16, isOutput=False)
    b0 = nc.declare_dram_parameter("b0", [MM], f32, isOutput=False)
    b1 = nc.declare_dram_parameter("b1", [MM], bf16, isOutput=False)
    WbS = nc.declare_dram_parameter("WbS", [C, S, QT], bf16, isOutput=False)
    bb = nc.declare_dram_parameter("bb", [C, S], bf16, isOutput=False)
    WoT = nc.declare_dram_parameter("WoT", [MM, OUT], bf16, isOutput=False)
    bout = nc.declare_dram_parameter("bout", [OUT], bf16, isOutput=False)
    out = nc.declare_dram_parameter("out", [BL, OUT], f32, isOutput=True)

    h0T_d = nc.dram_tensor("h0T_d", [MM, BL], bf16)
    zb_d = nc.dram_tensor("zb_d", [BL, MM], bf16)
    zn_d = nc.dram_tensor("zn_d", [BL, MM], bf16)

    # middle PSUM fill plan: 13 N-slices (12x512 + 256) packed into 5
    # [128,<=1536] psum tiles (3 banks each, double-buffered = 6 banks)
    FILLS = [(0, (512, 512, 512)), (1536, (512, 512, 512)),
             (3072, (512, 512, 512)), (4608, (512, 512, 512)),
             (6144, (256,))]

    with tile.TileContext(nc) as tc:
        with ExitStack() as top:
            const = top.enter_context(tc.tile_pool(name="const", bufs=1))
            h1p = top.enter_context(tc.tile_pool(name="h1b", bufs=1))
            ps_aux = top.enter_context(
                tc.tile_pool(name="ps_aux", bufs=2, space="PSUM"))

            identb = const.tile([128, 128], bf16)
            identf = const.tile([128, 128], f32)
            make_identity(nc, identf)
            nc.vector.tensor_copy(identb[:], identf[:])

            # b0 as per-partition bias columns: col mt holds m = mt*128 + p
            b0sb = const.tile([128, NMT], f32)
            nc.sync.dma_start(
                out=b0sb[:, : NMT - 1],
                in_=b0[: 128 * (NMT - 1)].rearrange("(j p) -> p j", p=128),
            )
            nc.sync.dma_start(
                out=b0sb[: _msz(NMT - 1), NMT - 1 : NMT],
                in_=b0[128 * (NMT - 1) :].unsqueeze(1),
            )
            b1rep = const.tile([128, MM], bf16)
            nc.sync.dma_start(
                out=b1rep[:], in_=b1[:].unsqueeze(0).broadcast_to([128, MM])
            )
            bbrep = const.tile([128, MM], bf16)
            nc.sync.dma_start(
                out=bbrep[:],
                in_=bb[:].rearrange("c q -> (c q)").unsqueeze(0).broadcast_to(
                    [128, MM]
                ),
            )
            borep = const.tile([128, OUT], bf16)
            nc.sync.dma_start(
                out=borep[:], in_=bout[:].unsqueeze(0).broadcast_to([128, OUT])
            )

            h1b = h1p.tile([128, NBT, MM], bf16)

            # ==== phase 1: A1 (h1b) + A0 (h0T) + middle, fully interleaved ===
            MSL = [(i * 256, 256) for i in range(6)] + [(1536, 64)]
            with ExitStack() as mctx:
                ps_mid = mctx.enter_context(
                    tc.tile_pool(name="ps_mid", bufs=2, space="PSUM"))
                xbp = mctx.enter_context(tc.tile_pool(name="a1x", bufs=1))
                w1p = mctx.enter_context(tc.tile_pool(name="a1w", bufs=2))
                e1p = mctx.enter_context(tc.tile_pool(name="a1e", bufs=2))
                xap = mctx.enter_context(tc.tile_pool(name="a0x", bufs=1))
                w0p = mctx.enter_context(tc.tile_pool(name="a0w", bufs=2))
                e0p = mctx.enter_context(tc.tile_pool(name="a0e", bufs=2))
                wbp = mctx.enter_context(tc.tile_pool(name="wb", bufs=2))
                h0p = mctx.enter_context(tc.tile_pool(name="h0c", bufs=2))
                y2p = mctx.enter_context(tc.tile_pool(name="y2", bufs=2))
                t1p = mctx.enter_context(tc.tile_pool(name="t1", bufs=2))
                t2p = mctx.enter_context(tc.tile_pool(name="t2", bufs=2))
                t3p = mctx.enter_context(tc.tile_pool(name="t3", bufs=2))
                zp = mctx.enter_context(tc.tile_pool(name="zst", bufs=2))

                xb = xbp.tile([128, NKT, BL], bf16)
                nc.sync.dma_start(
                    out=xb[:], in_=x1T[:].rearrange("(k p) b -> p k b", p=128)
                )
                xa = xap.tile([128, NKT, BL], bf16)
                nc.sync.dma_start(
                    out=xa[:], in_=x0T[:].rearrange("(k p) b -> p k b", p=128)
                )

                def emit_a1_ms(msi):
                    mo, mw = MSL[msi]
                    w1 = w1p.tile([128, NKT, 256], bf16, tag="w1")
                    nc.sync.dma_start(
                        out=w1[:, :, :mw],
                        in_=W1T[:, mo : mo + mw].rearrange(
                            "(k p) m -> p k m", p=128
                        ),
                    )
                    for bt in range(NBT):
                        ps = ps_aux.tile([128, 512], f32, tag="psa")
                        for k in range(NKT):
                            nc.tensor.matmul(
                                ps[:, :mw],
                                lhsT=xb[:, k, bt * 128 : (bt + 1) * 128],
                                rhs=w1[:, k, :mw],
                                start=(k == 0),
                                stop=(k == NKT - 1),
                            )
                        ev = e1p.tile([128, 256], bf16, tag="e1")
                        nc.scalar.activation(ev[:, :mw], ps[:, :mw], AF.Identity)
                        nc.vector.tensor_tensor(
                            out=h1b[:, bt, mo : mo + mw],
                            in0=ev[:, :mw],
                            in1=b1rep[:, mo : mo + mw],
                            op=ALU.add,
                        )

                def emit_a0_mt(mt):
                    ms = _msz(mt)
                    msl = slice(mt * 128, mt * 128 + ms)
                    w0 = w0p.tile([128, NKT, 128], bf16, tag="w0")
                    nc.sync.dma_start(
                        out=w0[:, :, :ms],
                        in_=W0T[:, msl].rearrange("(k p) m -> p k m", p=128),
                    )
                    for hh in range(2):
                        hsl = slice(hh * 512, (hh + 1) * 512)
                        ps = ps_aux.tile([128, 512], f32, tag="psa")
                        for k in range(NKT):
                            nc.tensor.matmul(
                                ps[:ms, :],
                                lhsT=w0[:, k, :ms],
                                rhs=xa[:, k, hsl],
                                start=(k == 0),
                                stop=(k == NKT - 1),
                            )
                        ev = e0p.tile([128, 512], bf16, tag="e0")
                        nc.scalar.activation(
                            ev[:ms, :], ps[:ms, :], AF.Identity,
                            bias=b0sb[:ms, mt : mt + 1],
                        )
                        nc.sync.dma_start(out=h0T_d[msl, hsl], in_=ev[:ms, :])

                def emit_chunk(c):
                    csl = slice(c * S, (c + 1) * S)
                    wbs = wbp.tile([S, QT], bf16, tag="wbs")
                    nc.sync.dma_start(out=wbs[:], in_=WbS[c])
                    h0c = h0p.tile([S, BL], bf16, tag="h0c")
                    nc.sync.dma_start(out=h0c[:], in_=h0T_d[csl, :])
                    for bt in range(NBT):
                        bsl = slice(bt * 128, (bt + 1) * 128)
                        y2 = y2p.tile([128, QT], bf16, tag="y2")
                        for off, ws in FILLS:
                            fw = sum(ws)
                            ps = ps_mid.tile([128, 1536], f32, tag="mid")
                            o = 0
                            for w in ws:
                                nc.tensor.matmul(
                                    ps[:, o : o + w],
                                    lhsT=h0c[:, bsl],
                                    rhs=wbs[:, off + o : off + o + w],
                                    start=True,
                                    stop=True,
                                )
                                o += w
                            if fw <= 256:
                                nc.vector.tensor_copy(
                                    y2[:, off : off + fw], ps[:, :fw]
                                )
                            else:
                                nc.scalar.activation(
                                    y2[:, off : off + fw], ps[:, :fw],
                                    AF.Identity,
                                )
                        y23 = y2[:].rearrange("p (q t) -> p q t", t=S)
                        nc.vector.tensor_tensor(
                            out=y23,
                            in0=y23,
                            in1=h1b[:, bt, csl].unsqueeze(1).broadcast_to(
                                [128, S, S]
                            ),
                            op=ALU.mult,
                        )
                        t1 = t1p.tile([128, S, 40], bf16, tag="t1")
                        nc.vector.tensor_tensor(
                            out=t1[:], in0=y23[:, :, :40],
                            in1=y23[:, :, 40:], op=ALU.add,
                        )
                        t2 = t2p.tile([128, S, 20], bf16, tag="t2")
                        eng2 = nc.gpsimd if (c * NBT + bt) % 10 < 7 else nc.vector
                        eng2.tensor_tensor(
                            out=t2[:], in0=t1[:, :, :20], in1=t1[:, :, 20:],
                            op=ALU.add,
                        )
                        t3 = t3p.tile([128, S, 10], bf16, tag="t3")
                        nc.gpsimd.tensor_tensor(
                            out=t3[:], in0=t2[:, :, :10], in1=t2[:, :, 10:],
                            op=ALU.add,
                        )
                        t4 = t3p.tile([128, S, 5], bf16, tag="t4")
                        nc.gpsimd.tensor_tensor(
                            out=t4[:], in0=t3[:, :, :5], in1=t3[:, :, 5:],
                            op=ALU.add,
                        )
                        t5 = t3p.tile([128, S, 2], bf16, tag="t5")
                        nc.gpsimd.tensor_tensor(
                            out=t5[:], in0=t4[:, :, 0:2], in1=t4[:, :, 2:4],
                            op=ALU.add,
                        )
                        t6 = zp.tile([128, S, 1], bf16, tag="t6")
                        nc.gpsimd.tensor_tensor(
                            out=t6[:], in0=t5[:, :, 0:1], in1=t5[:, :, 1:2],
                            op=ALU.add,
                        )
                        zst = zp.tile([128, S], bf16, tag="zst")
                        nc.gpsimd.tensor_tensor(
                            out=zst[:].unsqueeze(2),
                            in0=t6[:],
                            in1=t4[:, :, 4:5],
                            op=ALU.add,
                        )
                        nc.sync.dma_start(out=zb_d[bsl, csl], in_=zst[:])

                # chunk c is emittable once A0 covers rows 80c+79 (mt) and A1
                # covers cols 80c+79 (ms)
                def ms_hi(c):
                    return min(len(MSL) - 1, (80 * c + 79) // 256)

                def mt_hi(c):
                    return (80 * c + 79) // 128

                done = 0
                ms_done = 0
                emit_a1_ms(0)
                ms_done = 1
                for mt in range(NMT):
                    emit_a0_mt(mt)
                    ready = min(C, (128 * (mt + 1)) // 80)
                    while done < ready:
                        if ms_hi(done) >= ms_done:
                            emit_a1_ms(ms_done)
                            ms_done += 1
                        emit_chunk(done)
                        done += 1
                while ms_done < len(MSL):
                    emit_a1_ms(ms_done)
                    ms_done += 1
                while done < C:
                    emit_chunk(done)
                    done += 1

            # ==== phase 2: tail + znT + out-proj, per batch-tile ====
            with ExitStack() as octx:
                ps_out = octx.enter_context(
                    tc.tile_pool(name="ps_out", bufs=2, space="PSUM"))
                zlp = octx.enter_context(tc.tile_pool(name="tl", bufs=2))
                znpp = octx.enter_context(tc.tile_pool(name="zn", bufs=2))
                sp = octx.enter_context(tc.tile_pool(name="tls", bufs=2))
                znp = octx.enter_context(tc.tile_pool(name="znT", bufs=1))
                wop = octx.enter_context(tc.tile_pool(name="wo", bufs=1))
                evp = octx.enter_context(tc.tile_pool(name="oev", bufs=3))

                # all 6 Wout column groups resident (bf16, 80KB)
                wos = wop.tile([128, NOG, NMT, 512], bf16)
                for og in range(NOG):
                    ow = _osz(og)
                    osl = slice(og * 512, og * 512 + ow)
                    nc.sync.dma_start(
                        out=wos[:, og, : NMT - 1, :ow],
                        in_=WoT[: 128 * (NMT - 1), osl].rearrange(
                            "(k p) o -> p k o", p=128
                        ),
                    )
                    nc.sync.dma_start(
                        out=wos[: _msz(NMT - 1), og, NMT - 1, :ow],
                        in_=WoT[128 * (NMT - 1) :, osl],
                    )

                znT = znp.tile([128, NMT, 128], bf16)  # one bt at a time
                for bt in range(NBT):
                    bsl = slice(bt * 128, (bt + 1) * 128)
                    zt = zlp.tile([128, MM], bf16, tag="zt")
                    nc.sync.dma_start(out=zt[:], in_=zb_d[bsl, :])
                    zbb = zlp.tile([128, MM], bf16, tag="zbb")
                    nc.vector.tensor_tensor(
                        out=zbb[:], in0=zt[:], in1=bbrep[:], op=ALU.add
                    )
                    sab = zlp.tile([128, MM], bf16, tag="zt")
                    nc.scalar.activation(sab[:], zbb[:], AF.Abs)
                    sq = zlp.tile([128, MM], bf16, tag="sq")
                    nc.scalar.activation(sq[:], sab[:], AF.Sqrt)
                    sgn = zlp.tile([128, MM], bf16, tag="sgn")
                    nc.scalar.activation(sgn[:], zbb[:], AF.Sign)
                    ss = zlp.tile([128, MM], bf16, tag="ss")
                    nc.vector.tensor_tensor(
                        out=ss[:], in0=sgn[:], in1=sq[:], op=ALU.mult
                    )
                    nsq = sp.tile([128, C], f32, tag="nsq")
                    nc.vector.tensor_reduce(
                        out=nsq[:],
                        in_=zbb[:].rearrange("p (c q) -> p c q", q=S),
                        axis=AX.X,
                        op=ALU.add,
                        apply_absolute_value=True,
                    )
                    nrm = sp.tile([128, C], f32, tag="nrm")
                    nc.scalar.activation(nrm[:], nsq[:], AF.Sqrt)
                    nrmc = sp.tile([128, C], f32, tag="nrmc")
                    nc.vector.tensor_scalar_max(
                        out=nrmc[:], in0=nrm[:], scalar1=EPS
                    )
                    inv = sp.tile([128, C], f32, tag="inv")
                    nc.vector.reciprocal(inv[:], nrmc[:])
                    zn = znpp.tile([128, MM], bf16, tag="zn")
                    nc.vector.tensor_tensor(
                        out=zn[:].rearrange("p (c q) -> p c q", q=S),
                        in0=ss[:].rearrange("p (c q) -> p c q", q=S),
                        in1=inv[:].unsqueeze(2).broadcast_to([128, C, S]),
                        op=ALU.mult,
                    )
                    # transpose this bt into znT, then its 6 out-proj groups
                    for kq in range(4):
                        pst = ps_out.tile([128, 512], bf16, tag="pst")
                        nw = min(4, NMT - kq * 4)
                        for i in range(nw):
                            k = kq * 4 + i
                            ks = _msz(k)
                            nc.tensor.transpose(
                                pst[:ks, i * 128 : i * 128 + 128],
                                zn[:, k * 128 : k * 128 + ks],
                                identb[:],
                            )
                        nc.scalar.activation(
                            znT[:, kq * 4 : kq * 4 + nw, :],
                            pst[:, : nw * 128].rearrange(
                                "p (k b) -> p k b", b=128
                            ),
                            AF.Identity,
                        )
                    for og in range(NOG):
                        ow = _osz(og)
                        osl = slice(og * 512, og * 512 + ow)
                        ps = ps_out.tile([128, 512], f32, tag="acc")
                        for k in range(NMT):
                            ks = _msz(k)
                            nc.tensor.matmul(
                                ps[:, :ow],
                                lhsT=znT[:ks, k, :],
                                rhs=wos[:ks, og, k, :ow],
                                start=(k == 0),
                                stop=(k == NMT - 1),
                            )
                        ev = evp.tile([128, 512], f32, tag="oev")
                        nc.vector.tensor_tensor(
                            out=ev[:, :ow], in0=ps[:, :ow], in1=borep[:, osl],
                            op=ALU.add,
                        )
                        nc.sync.dma_start(out=out[bsl, osl], in_=ev[:, :ow])

    _split_excess_waits(nc, cap=4)
    return nc


def _split_excess_waits(nc, cap=4):
    """Walrus rejects instructions with too many sync waits. Move excess
    waits onto NoOps spliced just before the instruction on the same engine
    queue (the sequencer executes them in order, so semantics are identical).
    """
    import concourse.mybir as mybir
    import bass_rust

    n = 0
    for f in nc.m.functions:
        for blk in f.blocks:
            out = []
            changed = False
            for inst in blk.instructions:
                si = getattr(inst, "sync_info", None)
                waits = list(si.on_wait) if si is not None and si.on_wait else []
                icap = 2 if inst.opcode == "EventSemaphore" else 1
                if len(waits) > icap:
                    excess, keep = waits[:-icap], waits[-icap:]
                    for w in excess:
                        nop = mybir.InstNoOp(
                            name=f"{inst.name}-wsplit{n}", ins=[], outs=[]
                        )
                        n += 1
                        nop.engine = inst.engine
                        nop.sync_info = bass_rust.SyncInfo(
                            on_wait=[w], on_update=[]
                        )
                        out.append(nop)
                    inst.sync_info = bass_rust.SyncInfo(
                        on_wait=keep, on_update=list(si.on_update or [])
                    )
                    changed = True
                out.append(inst)
            if changed:
                blk.instructions = out
    return nc


def _get_nc():
    if "nc" not in _CACHE:
        _CACHE["nc"] = _build()
    return _CACHE["nc"]


def _shard_inputs(inputs):
    import ml_dtypes

    bf = ml_dtypes.bfloat16
    full = {k: np.asarray(v, dtype=np.float32) for k, v in inputs.items()}
    shared = {
        "W0T": np.ascontiguousarray(full["W0"].T.astype(bf)),
        "W1T": np.ascontiguousarray(full["W1"].T.astype(bf)),
        "WbS": np.ascontiguousarray(
            full["Wb"].transpose(0, 2, 1, 3).reshape(C, S, QT).astype(bf)
        ),
        "WoT": np.ascontiguousarray(full["Wout"].T.astype(bf)),
        "b0": np.ascontiguousarray(full["b0"]),
        "b1": np.ascontiguousarray(full["b1"].astype(bf)),
        "bb": np.ascontiguousarray(full["bb"].astype(bf)),
        "bout": np.ascontiguousarray(full["bout"].astype(bf)),
    }
    rows = full["x0"].shape[0] // NCORES
    in_maps = []
    for i in range(NCORES):
        m = dict(shared)
        m["x0T"] = np.ascontiguousarray(
            full["x0"][i * rows : (i + 1) * rows].T.astype(bf)
        )
        m["x1T"] = np.ascontiguousarray(
            full["x1"][i * rows : (i + 1) * rows].T.astype(bf)
        )
        in_maps.append(m)
    return in_maps


def kernel(**inputs):
    from concourse.bass_utils import run_bass_kernel_spmd

    nc = _get_nc()
    in_maps = _shard_inputs(inputs)
    res = run_bass_kernel_spmd(nc, in_maps, list(range(NCORES)))
    return np.concatenate([res.results[i]["out"] for i in range(NCORES)], axis=0)
